# revision 1
# baseline (speedup 1.0000x reference)
"""Trainium2 Bass kernel for nn_CamMemory (soft cross-entropy vs. memory bank).

Computes: x = normalize(inputs); logits = x @ features.T / TEMP;
loss = mean_b( lse(logits_b) - dot(softmax(targets_b), logits_b) )

Sharding: features/targets split row-wise (N dim) across 8 cores; inputs
replicated.  Each core returns partial stats (s, u, p) per batch row:
  s = sum_n exp(logits - SHIFT)      (partial sum-exp, fixed shift; |logits|<=21)
  u = sum_n exp(targets - 1)         (partial softmax denominator; targets in [0,1))
  p = sum_n exp(targets - 1)*logits  (partial weighted logit sum)
Host combines: loss = mean_b( SHIFT + log(sum s) - (sum p)/(sum u) ).

Per-core pipeline (DMA budget is the 16.8MB feature load; everything else
stays off the DMA subsystem):
  - SWDGE cast-DMA features f32 DRAM -> bf16 SBUF, natural layout (n on
    partitions), 1MB chunks.
  - PE transpose-mode matmuls flip each 128x128 block into PSUM staging
    banks (8 blocks per bank), putting D on partitions.
  - Batched PSUM->SBUF copies (DVE/ACT alternating) build featT chunks.
  - bf16 matmuls with xT (DMA-xbar transposed, 1/TEMP and 1/||x|| folded in)
    stationary accumulate logits [64, 128] per chunk.
  - Fused exp+row-sum on ACT; mul+row-sum on DVE.
"""

import numpy as np

import concourse.bacc as bacc
import concourse.mybir as mybir
import concourse.tile as tile
from concourse.masks import make_identity
from concourse.tile_rust import add_dep_helper

B = 64
D = 2048
N = 16384
NUM_CORES = 8
NSH = N // NUM_CORES  # 2048 rows of features per core
TEMP = 0.05
SHIFT = 21.0  # |logits| <= (1/TEMP)*|x.f| <= 20*(1+eps) since both unit-norm

F32 = mybir.dt.float32
BF16 = mybir.dt.bfloat16


def build_nc(d=D, nsh=NSH, b=B, debug=False):
    """Build the single-core Bass program (SPMD: same program, 8 shards)."""
    kc = d // 128     # contraction chunks (d on partitions)
    nch = nsh // 128  # feature-row chunks
    TG = min(8, kc)   # transposed blocks staged per PSUM bank
    ngrp = kc // TG

    nc = bacc.Bacc("TRN2", target_bir_lowering=False, debug=debug)

    inputs_d = nc.dram_tensor("inputs", [b, d], F32, kind="ExternalInput")
    targets_d = nc.dram_tensor("targets", [b, nsh], F32, kind="ExternalInput")
    features_d = nc.dram_tensor("features", [nsh, d], F32, kind="ExternalInput")
    out_d = nc.dram_tensor("out", [b, 4], F32, kind="ExternalOutput")

    with tile.TileContext(nc) as tc:
        with (
            tc.tile_pool(name="small", bufs=1) as small,
            tc.tile_pool(name="nat", bufs=6) as natp,
            tc.tile_pool(name="ft", bufs=4) as ftp,
            tc.tile_pool(name="epi", bufs=4) as epi,
            tc.tile_pool(name="tps", bufs=4, space="PSUM") as tpsp,
            tc.tile_pool(name="psum", bufs=4, space="PSUM") as psp,
        ):
            # constants
            ident = small.tile([128, 128], BF16)
            make_identity(nc, ident[:])
            bias_m1 = small.tile([b, 1], F32)
            nc.vector.memset(bias_m1[:], -1.0)
            bias_shift = small.tile([b, 1], F32)
            nc.vector.memset(bias_shift[:], -float(SHIFT))

            # HAM pre-warm: ~40 throwaway matmuls while the PE waits for the
            # first cast-DMA, so the clock gate is at 8/8 (2.4GHz) before the
            # real transposes/matmuls start (saves the ~10us cold window).
            dwarm = psp.tile([b, 128], F32, tag="ps")
            for _ in range(40):
                nc.tensor.matmul(dwarm[:], ident[:, 0:b], ident[:],
                                 start=True, stop=True)

            # ---- x preparation: x = (inputs/||inputs||) / TEMP, bf16, transposed
            xin = small.tile([b, d], F32)
            nc.sync.dma_start(xin[:], inputs_d[:])
            sq = small.tile([b, d], F32)
            ss = small.tile([b, 1], F32)
            nc.scalar.activation(
                sq[:], xin[:], mybir.ActivationFunctionType.Square,
                accum_out=ss[:],
            )
            # inv = (1/TEMP)/sqrt(ss):  sqrt(ss*TEMP^2) then reciprocal
            srt = small.tile([b, 1], F32)
            i_sqrt = nc.scalar.activation(
                srt[:], ss[:], mybir.ActivationFunctionType.Sqrt,
                scale=float(TEMP) * float(TEMP),
            )
            inv = small.tile([b, 1], F32)
            nc.vector.reciprocal(inv[:], srt[:])
            # x padded to 128 partitions so its PE transposes exactly mirror
            # the feature-block pattern (a DMA-xbar transpose here would
            # force a full DMA-pipeline flush against the streaming casts)
            xbp = small.tile([128, d], BF16)
            nc.gpsimd.memset(xbp[b:128, :], 0.0)
            i_tsmul = nc.vector.tensor_scalar_mul(xbp[:b, :], xin[:], inv[:])
            xT = small.tile([128, kc, 128], BF16)

            # ---- targets: exp(t - 1) and its row-sum u
            tg = small.tile([b, nsh], F32)
            nc.sync.dma_start(tg[:], targets_d[:])
            et = small.tile([b, nsh], F32)
            u = small.tile([b, 1], F32)
            i_etexp = nc.scalar.activation(
                et[:], tg[:], mybir.ActivationFunctionType.Exp,
                bias=bias_m1[:], accum_out=u[:],
            )
            # et-exp must not preempt the x-chain on ACT
            add_dep_helper(i_etexp.ins, i_sqrt.ins, sync=False,
                           reason="x-chain first on ACT")

            # x transposes through the same PSUM staging pool as features
            for g in range(ngrp):
                tp = tpsp.tile([128, TG, 128], BF16)
                for j in range(TG):
                    k = g * TG + j
                    nc.tensor.transpose(
                        tp[:, j, :], xbp[:, k * 128:(k + 1) * 128], ident[:])
                i_xcp = nc.vector.tensor_copy(xT[:, g * TG:(g + 1) * TG, :], tp[:])
                add_dep_helper(i_xcp.ins, i_tsmul.ins, sync=False,
                               reason="x-chain first on DVE")

            # ---- features pipeline: per 128-row chunk, software-pipelined
            # by one chunk so the logits matmuls of chunk c-1 run while the
            # PSUM->SBUF copies of chunk c are still in flight (the PE never
            # sits waiting on a copy it just enabled).
            s_parts = small.tile([b, nch], F32)
            p_parts = small.tile([b, nch], F32)

            def emit_mm(prev, k):
                pc, pftc, pps = prev
                nc.tensor.matmul(
                    pps[:], xT[:, k, 0:b], pftc[:, k, :],
                    start=(k == 0), stop=(k == kc - 1),
                )

            def emit_epi(prev):
                pc, pftc, pps = prev
                # s_part = sum_n exp(logits - SHIFT)   (fused on ACT)
                el = epi.tile([b, 128], F32)
                nc.scalar.activation(
                    el[:], pps[:], mybir.ActivationFunctionType.Exp,
                    bias=bias_shift[:], accum_out=s_parts[:, pc:pc + 1],
                )
                # p_part = sum_n exp_t * logits        (DVE mul + reduce)
                pm = epi.tile([b, 128], F32)
                nc.vector.tensor_mul(pm[:], et[:, pc * 128:(pc + 1) * 128], pps[:])
                nc.vector.reduce_sum(
                    p_parts[:, pc:pc + 1], pm[:], axis=mybir.AxisListType.X)

            prev = None
            for c in range(nch):
                natc = natp.tile([128, d], BF16)
                # SWDGE cast-DMA: f32 DRAM -> bf16 SBUF (the only big DMA)
                nc.gpsimd.dma_start(natc[:], features_d[c * 128:(c + 1) * 128, :])

                # PE transposes 128x128 blocks into PSUM staging; batched
                # copies move them to SBUF as featT [128(d), kc, 128(n)].
                # Chunk c-1's logits matmuls interleave 1:1 with chunk c's
                # transposes: real MMs land in every HAM window (transpose-
                # mode ops don't count as PE-busy), keeping the PE at 2.4GHz.
                ftc = ftp.tile([128, kc, 128], BF16)
                for g in range(ngrp):
                    tp = tpsp.tile([128, TG, 128], BF16)
                    for j in range(TG):
                        k = g * TG + j
                        nc.tensor.transpose(
                            tp[:, j, :], natc[:, k * 128:(k + 1) * 128], ident[:])
                    dst = ftc[:, g * TG:(g + 1) * TG, :]
                    if True:
                        i_cp = nc.vector.tensor_copy(dst, tp[:])
                        if c < 4:
                            # copies must not preempt the x-chain on DVE
                            add_dep_helper(i_cp.ins, i_tsmul.ins, sync=False,
                                           reason="x-chain first on DVE")
                    else:
                        i_cp = nc.scalar.copy(dst, tp[:])
                        if c < 4:
                            add_dep_helper(i_cp.ins, i_sqrt.ins, sync=False,
                                           reason="x-chain first on ACT")

                if prev is not None:
                    for k in range(kc):
                        emit_mm(prev, k)
                    emit_epi(prev)
                ps = psp.tile([b, 128], F32)
                prev = (c, ftc, ps)
            for k in range(kc):
                emit_mm(prev, k)
            emit_epi(prev)

            # ---- final per-core reduction and output
            sbout = small.tile([b, 4], F32)
            nc.vector.reduce_sum(
                sbout[:, 0:1], s_parts[:], axis=mybir.AxisListType.X)
            nc.vector.tensor_copy(sbout[:, 1:2], u[:])
            nc.vector.reduce_sum(
                sbout[:, 2:3], p_parts[:], axis=mybir.AxisListType.X)
            nc.vector.memset(sbout[:, 3:4], 0.0)
            nc.sync.dma_start(out_d[:], sbout[:])

    nc.compile()
    return nc


_NC_CACHE = None


def _run(inputs, trace=False, **spmd_kwargs):
    global _NC_CACHE
    from concourse.bass_utils import run_bass_kernel_spmd

    x = np.ascontiguousarray(np.asarray(inputs["inputs"], dtype=np.float32))
    t = np.asarray(inputs["targets"], dtype=np.float32)
    f = np.asarray(inputs["features"], dtype=np.float32)
    # cid is unused by the reference computation.

    if _NC_CACHE is None:
        _NC_CACHE = build_nc(debug=False)
    nc = _NC_CACHE

    in_maps = []
    for c in range(NUM_CORES):
        in_maps.append({
            "inputs": x,
            "targets": np.ascontiguousarray(t[:, c * NSH:(c + 1) * NSH]),
            "features": np.ascontiguousarray(f[c * NSH:(c + 1) * NSH, :]),
        })

    res = run_bass_kernel_spmd(
        nc, in_maps, core_ids=list(range(NUM_CORES)), trace=trace, **spmd_kwargs)
    outs = np.stack([r["out"] for r in res.results])  # [8, B, 4]

    outs64 = outs.astype(np.float64)
    s = outs64[:, :, 0].sum(0)
    u = outs64[:, :, 1].sum(0)
    p = outs64[:, :, 2].sum(0)
    lse = SHIFT + np.log(s)
    loss = np.mean(lse - p / u)
    return np.float32(loss), res


def kernel(**inputs: np.ndarray) -> np.ndarray:
    loss, _ = _run(inputs)
    return np.asarray(loss, dtype=np.float32)



# revision 19
# speedup vs baseline: 1.0465x; 1.0465x over previous
"""Trainium2 Bass kernel for nn_CamMemory (soft cross-entropy vs. memory bank).

Computes: x = normalize(inputs); logits = x @ features.T / TEMP;
loss = mean_b( lse(logits_b) - dot(softmax(targets_b), logits_b) )

Sharding: features/targets split row-wise (N dim) across 8 cores; inputs
replicated.  Each core returns partial stats (s, p, u) per batch row:
  s = sum_n exp(logits - SHIFT)      (partial sum-exp, fixed shift; |logits|<=21)
  p = sum_n exp(targets - 1)*logits  (partial weighted logit sum)
  u = sum_n exp(targets - 1)         (partial softmax denominator; targets in [0,1))
Host combines: loss = mean_b( SHIFT + log(sum s) - (sum p)/(sum u) ).

Per-core pipeline (DMA budget is the 16.8MB feature load; PE stream work is
kept at ~23us so it hides fully under the ~44us wire time):
  - SWDGE cast-DMA features f32 DRAM -> bf16 SBUF, natural layout (n on
    partitions), 1MB chunks, issued as the FIRST gpsimd work.
  - PE transpose-mode flips 128x128 blocks into PSUM (d on partitions);
    DVE copies batches of 8 blocks to SBUF featT.
  - Matmuls use featT blocks as STATIONARY and xT (64 cols) as MOVING ->
    logitsT [128n, 64b] in PSUM, 64 cycles each (half the moving cycles of
    the b-major orientation).
  - Epilogue per chunk with n on partitions: ACT exp -> el (bf16), DVE
    et*logits -> pm (bf16); s/p/u reduced over n by accumulating
    ones-matmuls into one PSUM bank ([1,192]).
"""

import numpy as np

import concourse.bacc as bacc
import concourse.mybir as mybir
import concourse.tile as tile
from concourse.masks import make_identity

B = 64
D = 2048
N = 16384
NUM_CORES = 8
NSH = N // NUM_CORES  # 2048 rows of features per core
TEMP = 0.05
SHIFT = 21.0  # |logits| <= (1/TEMP)*|x.f| <= 20*(1+eps) since both unit-norm

F32 = mybir.dt.float32
BF16 = mybir.dt.bfloat16


def build_nc(d=D, nsh=NSH, b=B, debug=False, no_epi=False, no_ones=False,
             sb_et=False, no_mm=False, no_xchain=False, no_warm=False,
             no_tr=False, no_small_tr=False):
    """Build the single-core Bass program (SPMD: same program, 8 shards)."""
    kc = d // 128     # contraction chunks (d on partitions)
    nch = nsh // 128  # feature-row chunks
    TG = 8            # transposed blocks staged per PSUM bank
    ngrp = kc // TG
    NWARM = 32

    nc = bacc.Bacc("TRN2", target_bir_lowering=False, debug=debug)

    inputs_d = nc.dram_tensor("inputs", [b, d], F32, kind="ExternalInput")
    targets_d = nc.dram_tensor("targets", [b, nsh], F32, kind="ExternalInput")
    features_d = nc.dram_tensor("features", [nsh, d], F32, kind="ExternalInput")
    out_d = nc.dram_tensor("out", [1, 192], F32, kind="ExternalOutput")

    with tile.TileContext(nc) as tc:
        with (
            tc.tile_pool(name="small", bufs=1) as small,
            tc.tile_pool(name="nat", bufs=6) as natp,
            tc.tile_pool(name="ft", bufs=3) as ftp,
            tc.tile_pool(name="epi", bufs=3) as epi,
            tc.tile_pool(name="tps", bufs=3, space="PSUM") as tpsp,
            tc.tile_pool(name="lps", bufs=3, space="PSUM") as lpsp,
            tc.tile_pool(name="spu", bufs=2, space="PSUM") as spup,
        ):
            # ---- feature cast-DMAs first: gpsimd's first instructions are
            # dma_starts so HBM streaming begins as early as possible.
            natcs = []
            for c in range(nch):
                natc = natp.tile([128, d], BF16, tag="nat")
                nc.gpsimd.dma_start(natc[:], features_d[c * 128:(c + 1) * 128, :])
                natcs.append(natc)
                if c == 1:
                    # identity (gpsimd memset+affine) between the first two
                    # issues and the rest; needed by warmup/transposes ~7.5us.
                    ident = small.tile([128, 128], BF16)
                    make_identity(nc, ident[:])

            # constants (DVE memsets; do not touch gpsimd)
            bias_m1 = small.tile([128, 1], F32)
            nc.vector.memset(bias_m1[:], -1.0)
            bias_shift = small.tile([128, 1], F32)
            nc.vector.memset(bias_shift[:], -float(SHIFT))
            ones = small.tile([128, 1], BF16)
            nc.vector.memset(ones[:], 1.0)

            # HAM pre-warm: throwaway matmuls while the first cast-DMA is in
            # flight, so the PE clock gate is 8/8 (2.4GHz) for the real work.
            if not no_warm:
                dwarm = lpsp.tile([128, 64], F32, tag="lp")
                for _ in range(NWARM):
                    nc.tensor.matmul(dwarm[:], ident[:], ident[:, 0:64],
                                     start=True, stop=True)

            # ---- x chain: xn = (inputs/||inputs||)/TEMP as bf16, transposed
            if no_xchain:
                no_mm = True
            xT = small.tile([128, kc, 64], BF16)
            if not no_xchain:
                xin = small.tile([b, d], F32)
                nc.sync.dma_start(xin[:], inputs_d[:])
                sq = small.tile([b, d], F32)
                ss = small.tile([b, 1], F32)
                nc.scalar.activation(
                    sq[:], xin[:], mybir.ActivationFunctionType.Square,
                    accum_out=ss[:],
                )
                srt = small.tile([b, 1], F32)
                nc.scalar.activation(
                    srt[:], ss[:], mybir.ActivationFunctionType.Sqrt,
                    scale=float(TEMP) * float(TEMP),
                )
                inv = small.tile([b, 1], F32)
                nc.vector.reciprocal(inv[:], srt[:])
                xnb = small.tile([b, d], BF16)
                nc.vector.tensor_scalar_mul(xnb[:], xin[:], inv[:])
                # transpose x: 16 blocks [64,128] -> [128,64]
                for g in range(0 if no_small_tr else ngrp):
                    tpx = tpsp.tile([128, TG, 128], BF16, tag="tps")
                    for j in range(TG):
                        k = g * TG + j
                        nc.tensor.transpose(
                            tpx[:, j, 0:b], xnb[:, k * 128:(k + 1) * 128],
                            ident[0:b, 0:b])
                    nc.vector.tensor_copy(xT[:, g * TG:(g + 1) * TG, :],
                                          tpx[:, :, 0:b])

            # ---- targets chain: etT = exp(targets - 1), n on partitions
            etT = small.tile([128, nch, b], BF16)
            if not no_xchain:
                tg = small.tile([b, nsh], F32)
                nc.sync.dma_start(tg[:], targets_d[:])
                tgb = small.tile([b, nsh], BF16)
                nc.vector.tensor_copy(tgb[:], tg[:])
                for g in range(0 if no_small_tr else (nch // TG)):
                    tpt = tpsp.tile([128, TG, 128], BF16, tag="tps")
                    for j in range(TG):
                        c = g * TG + j
                        nc.tensor.transpose(
                            tpt[:, j, 0:b], tgb[:, c * 128:(c + 1) * 128],
                            ident[0:b, 0:b])
                    if sb_et:
                        ttmp = small.tile([128, TG, 64], BF16, tag="ttmp")
                        nc.vector.tensor_copy(ttmp[:], tpt[:, :, 0:b])
                        nc.scalar.activation(
                            etT[:, g * TG:(g + 1) * TG, :], ttmp[:],
                            mybir.ActivationFunctionType.Exp, bias=bias_m1[:])
                    else:
                        # exp(t-1) fused with the PSUM->SBUF move on ACT
                        nc.scalar.activation(
                            etT[:, g * TG:(g + 1) * TG, :], tpt[:, :, 0:b],
                            mybir.ActivationFunctionType.Exp, bias=bias_m1[:])

            # ---- s/p/u partials: per-chunk ones-matmul [1,192] (contiguous
            # start/stop groups), accumulated on DVE into SBUF.
            acc = small.tile([1, 192], F32)
            nc.vector.memset(acc[:], 0.0)

            def emit_epi(prev):
                if no_epi:
                    return
                pc, plps = prev
                el = epi.tile([128, 64], BF16, tag="el")
                nc.scalar.activation(
                    el[:], plps[:], mybir.ActivationFunctionType.Exp,
                    bias=bias_shift[:])
                pm = epi.tile([128, 64], BF16, tag="pm")
                nc.vector.tensor_mul(pm[:], etT[:, pc, :], plps[:])
                if no_ones:
                    return
                spu = spup.tile([1, 192], F32, tag="spu")
                nc.tensor.matmul(spu[:, 0:64], ones[:], el[:],
                                 start=True, stop=True)
                nc.tensor.matmul(spu[:, 64:128], ones[:], pm[:],
                                 start=True, stop=True)
                nc.tensor.matmul(spu[:, 128:192], ones[:], etT[:, pc, :],
                                 start=True, stop=True)
                nc.vector.tensor_add(acc[:], acc[:], spu[:])

            # ---- feature pipeline: per 128-row chunk, transposes feed
            # featT; logits matmuls use featT blocks stationary, xT moving.
            # Epilogue of chunk c-1 is emitted between chunk c's transposes
            # and matmuls so the PE never waits on ACT/DVE.
            prev = None
            for c in range(nch):
                natc = natcs[c]
                ftc = ftp.tile([128, kc, 128], BF16)
                if not no_tr:
                    for g in range(ngrp):
                        tp = tpsp.tile([128, TG, 128], BF16, tag="tps")
                        for j in range(TG):
                            k = g * TG + j
                            nc.tensor.transpose(
                                tp[:, j, :], natc[:, k * 128:(k + 1) * 128],
                                ident[:])
                        nc.vector.tensor_copy(ftc[:, g * TG:(g + 1) * TG, :],
                                              tp[:])
                else:
                    nc.vector.tensor_copy(ftc[:, 0, :], natc[:, 0:128])

                if prev is not None:
                    emit_epi(prev)
                if not no_mm:
                    lps = lpsp.tile([128, 64], F32, tag="lp")
                    for k in range(kc):
                        nc.tensor.matmul(
                            lps[:], ftc[:, k, :], xT[:, k, :],
                            start=(k == 0), stop=(k == kc - 1),
                        )
                    prev = (c, lps)
            if prev is not None:
                emit_epi(prev)

            # ---- output
            nc.sync.dma_start(out_d[:], acc[:])

    nc.compile()
    return nc


_NC_CACHE = None


def _run(inputs, trace=False, **spmd_kwargs):
    global _NC_CACHE
    from concourse.bass_utils import run_bass_kernel_spmd

    x = np.ascontiguousarray(np.asarray(inputs["inputs"], dtype=np.float32))
    t = np.asarray(inputs["targets"], dtype=np.float32)
    f = np.asarray(inputs["features"], dtype=np.float32)
    # cid is unused by the reference computation.

    if _NC_CACHE is None:
        _NC_CACHE = build_nc(debug=False)
    nc = _NC_CACHE

    in_maps = []
    for c in range(NUM_CORES):
        in_maps.append({
            "inputs": x,
            "targets": np.ascontiguousarray(t[:, c * NSH:(c + 1) * NSH]),
            "features": np.ascontiguousarray(f[c * NSH:(c + 1) * NSH, :]),
        })

    res = run_bass_kernel_spmd(
        nc, in_maps, core_ids=list(range(NUM_CORES)), trace=trace, **spmd_kwargs)
    outs = np.stack([r["out"] for r in res.results])  # [8, 1, 192]

    outs64 = outs.astype(np.float64).reshape(NUM_CORES, 192)
    s = outs64[:, 0:64].sum(0)
    p = outs64[:, 64:128].sum(0)
    u = outs64[:, 128:192].sum(0)
    lse = SHIFT + np.log(s)
    loss = np.mean(lse - p / u)
    return np.float32(loss), res


def kernel(**inputs: np.ndarray) -> np.ndarray:
    loss, _ = _run(inputs)
    return np.asarray(loss, dtype=np.float32)
